# revision 36
# baseline (speedup 1.0000x reference)
"""Two-layer GCN on 8 Trainium2 NeuronCores (Bass/Tile).

Math (reference, per layer):
    deg  = segment_sum(ones, dst)                 # target-side degrees
    dinv = where(deg>0, rsqrt(deg), 0)
    out[d] = dinv[d] * sum_{e: dst[e]=d} dinv[src[e]] * x[src[e]]  @ W  + b
(the x@W GEMM commutes with the segment sum, so we aggregate raw features
and apply W once per 128-node output block).

Distribution: dst nodes (and their incident edges) are sharded across the 8
cores; x (fp16) is replicated in every core's HBM.  Per 128-edge chunk the
kernel gathers the source rows with dma_gather, builds a dinv-weighted
one-hot selection matrix [128e x 128dst] on DVE, and scatter-adds via a
TensorE matmul accumulating in PSUM: psum[f, d] += gathered.T @ sel.  A
second matmul applies the layer weight; the layer-1 activations are
exchanged with an AllGather so layer 2 can gather any source row.

dma_gather indices are int16, so sources are split into lo (< 32768) and
hi (>= 32768) edge lists; the hi gather reads from a view of the feature
table offset by 32768 rows.
"""

import os
import sys
import time

sys.path.insert(0, "/opt/trn_rl_repo")

import numpy as np

import concourse.bass as bass
import concourse.bacc as bacc
import concourse.tile as tile
from concourse import mybir
from concourse.bass_utils import run_bass_kernel_spmd

P = 128
N_NODES = 50000
N_EDGES = 800000
IN_DIM = 128
HID_DIM = 128
OUT_DIM = 64
NCORES = 8
SHARD = N_NODES // NCORES          # 6250
NBLK = (SHARD + P - 1) // P        # 49 dst blocks per core (48 full + 106)
SPLIT = 32768                      # int16 index limit
SB_BLOCKS = 7                      # dst blocks per superblock (gather batch)
LAST_ROWS = SHARD - (NBLK - 1) * P # rows in the final dst block

# Filled by kernel() on the last run (for test.py introspection).
LAST_RESULTS = None


# --------------------------------------------------------------------------
# Host-side preprocessing
# --------------------------------------------------------------------------

def _wrap_idx(idx_chunks):
    """int16 indices for one dma_gather call: [16, n/16] wrap replicated to
    128 partitions.  idx_chunks: int array [n_chunks, 128]."""
    flat = idx_chunks.reshape(-1)
    n = flat.shape[0]
    arr = flat.reshape(n // 16, 16).T.astype(np.int16)   # [16, n/16]
    return np.tile(arr, (8, 1))                          # [128, n/16]


def preprocess(x, edge_index, W1, b1, W2, b2):
    x = np.asarray(x, dtype=np.float32)
    edge_index = np.asarray(edge_index).astype(np.int64)
    src_g = edge_index[0].astype(np.int32)
    dst_g = edge_index[1].astype(np.int32)

    deg = np.bincount(dst_g, minlength=N_NODES).astype(np.float32)
    dinv = np.where(deg > 0, 1.0 / np.sqrt(np.maximum(deg, 1.0)), 0.0).astype(
        np.float32
    )

    # per (core, block, lo/hi) edge lists
    owner = dst_g // SHARD
    blk_loc = (dst_g % SHARD) // P
    rel = (dst_g % SHARD) % P

    # superblock structure (gather batching) + the 3-group exchange layout:
    # layer-1 activations are exchanged with three AllGathers over superblock
    # groups [0-1], [2-4], [5-6], each landing in its own Shared tile while
    # later superblocks still compute.  Each group spans < 32768 rows of the
    # [group][core][row] permuted table, so layer 2 gathers straight from the
    # Shared tiles with one int16 index stream per group (no lo/hi split).
    sbs = [list(range(s, min(s + SB_BLOCKS, NBLK))) for s in range(0, NBLK, SB_BLOCKS)]
    sb_rows = []
    for sb in sbs:
        r0 = sb[0] * P
        r1 = min(sb[-1] * P + P, SHARD)
        sb_rows.append((r0, r1 - r0))
    GROUP_SBS = [(0, 2), (2, 5), (5, 7)]   # sb index ranges per group
    groups = []
    for (s0, s1) in GROUP_SBS:
        r0 = sb_rows[s0][0]
        r1 = sb_rows[s1 - 1][0] + sb_rows[s1 - 1][1]
        groups.append((r0, r1 - r0))
    pos_base = np.cumsum([0] + [NCORES * rows for (_, rows) in groups])[:-1]
    # pos[n] for n = c*SHARD + r, group-major then core-major
    pos = np.empty(N_NODES, np.int64)
    base = 0
    for (r0, rows) in groups:
        for c in range(NCORES):
            n0 = c * SHARD + r0
            pos[n0 : n0 + rows] = np.arange(base + c * rows, base + (c + 1) * rows)
        base += NCORES * rows

    def make_buckets(src_key, binof, nbins):
        key = ((owner * NBLK + blk_loc) * nbins + binof).astype(np.int64)
        order = np.argsort(key, kind="stable")
        key_s = key[order]
        src_s = src_key[order]
        rel_s = rel[order]
        bounds = np.searchsorted(key_s, np.arange(NCORES * NBLK * nbins + 1))
        nchunks = (
            (bounds[1:] - bounds[:-1]).reshape(NCORES, NBLK, nbins) + P - 1
        ) // P
        cap = nchunks.max(axis=0)
        cap[:, 0] = np.maximum(cap[:, 0], 1)
        return (src_s, rel_s, bounds), cap

    bk1, cap = make_buckets(src_g, (src_g >= SPLIT).astype(np.int32), 2)
    pos_src = pos[src_g].astype(np.int32)
    binof2 = np.digitize(pos_src, pos_base[1:]).astype(np.int32)
    bk2, cap2 = make_buckets(pos_src, binof2, 3)

    meta = {
        "cap": cap,
        "cap2": cap2,
        "sbs": sbs,
        "sb_rows": sb_rows,
        "groups": groups,
        "group_sbs": GROUP_SBS,
        "pos_base": pos_base,
        "has_b1": bool(np.any(np.asarray(b1))),
        "has_b2": bool(np.any(np.asarray(b2))),
    }

    # per-core arrays.  x is pre-scaled by dinv[src] so the selection matrix
    # is a plain one-hot (single DVE op); padded lanes get dstrel=-1, which
    # matches no iota column and therefore contributes nothing.
    in_maps = []
    x16 = (dinv[:, None] * x).astype(np.float16)

    def core_arrays(c, bk, cap_l, bases):
        src_s, rel_s, bounds = bk
        H = cap_l.shape[1]
        totals = [int(cap_l[:, h].sum()) for h in range(H)]
        cum = [0]
        for t in totals:
            cum.append(cum[-1] + t)
        idx_arr = [np.zeros((totals[h], P), np.int32) for h in range(H)]
        m_dst = np.zeros((P, cum[-1]), np.float32)
        off = [0] * H
        for b in range(NBLK):
            for h in range(H):
                k = (c * NBLK + b) * H + h
                s_arr = src_s[bounds[k] : bounds[k + 1]]
                r_arr = rel_s[bounds[k] : bounds[k + 1]]
                n = s_arr.shape[0]
                ncap = int(cap_l[b, h])
                idxs = np.zeros(ncap * P, np.int32)
                idxs[:n] = s_arr - bases[h]
                d_arr = np.full(ncap * P, -1.0, np.float32)
                d_arr[:n] = r_arr
                o = off[h]
                idx_arr[h][o : o + ncap] = idxs.reshape(ncap, P)
                m_dst[:, cum[h] + o : cum[h] + o + ncap] = d_arr.reshape(ncap, P).T
                off[h] += ncap
        wrapped = []
        for h in range(H):
            o = 0
            cols = []
            for sb in sbs:
                n_h = int(cap_l[sb, h].sum())
                if n_h:
                    cols.append(_wrap_idx(idx_arr[h][o : o + n_h]))
                    o += n_h
            wrapped.append(
                np.concatenate(cols, axis=1) if cols else np.zeros((P, 8), np.int16)
            )
        return wrapped, m_dst.astype(np.float16)

    for c in range(NCORES):
        (idx_lo_w, idx_hi_w), m_dst16 = core_arrays(c, bk1, cap, [0, SPLIT])
        idx2_w, m_dst16_2 = core_arrays(c, bk2, cap2, list(pos_base))

        tmp = np.zeros(NBLK * P, np.float32)
        tmp[:SHARD] = dinv[c * SHARD : (c + 1) * SHARD]
        dinvd = tmp.reshape(NBLK, P).T.copy()   # [p, b] = dinv[c*SHARD + b*P + p]

        im = {
            "x16": x16,
            "idx_lo": idx_lo_w,
            "idx_hi": idx_hi_w,
            "idx_g0": idx2_w[0],
            "idx_g1": idx2_w[1],
            "idx_g2": idx2_w[2],
            "m_dst16": m_dst16,
            "m_dst16_2": m_dst16_2,
            "dinvd": dinvd,
            "dinvd2": dinvd * dinvd,
            "w1": np.asarray(W1, np.float32).astype(np.float16),
            "w2": np.asarray(W2, np.float32).astype(np.float16),
        }
        if meta["has_b1"]:
            im["b1rep"] = np.tile(np.asarray(b1, np.float32)[None, :], (P, 1))
        if meta["has_b2"]:
            im["b2rep"] = np.tile(np.asarray(b2, np.float32)[None, :], (P, 1))
        in_maps.append(im)
    return meta, in_maps


# --------------------------------------------------------------------------
# Bass kernel
# --------------------------------------------------------------------------

def build(meta):
    cap = meta["cap"]
    cap2 = meta["cap2"]
    sbs = meta["sbs"]
    sb_rows = meta["sb_rows"]
    groups = meta["groups"]
    total_lo = int(cap[:, 0].sum())
    total_hi = int(cap[:, 1].sum())
    tg = [int(cap2[:, h].sum()) for h in range(cap2.shape[1])]

    nc = bacc.Bacc(
        "TRN2",
        target_bir_lowering=False,
        debug=False,
        enable_asserts=True,
        num_devices=NCORES,
        num_swdge_queues=4,
        dynamic_dma_scratch_size=int(os.environ.get("GCN_DMA_SCRATCH", "16384")),
    )
    pipe = bool(int(os.environ.get("GCN_PIPE", "1")))
    ag_lag = int(os.environ.get("GCN_AG_LAG", "2"))
    gbufs = int(os.environ.get("GCN_GPOOL_BUFS", "3"))
    x16 = nc.dram_tensor("x16", [N_NODES, IN_DIM], mybir.dt.float16, kind="ExternalInput")
    idx_lo_d = nc.dram_tensor("idx_lo", [P, total_lo * 8], mybir.dt.int16, kind="ExternalInput")
    idx_hi_d = nc.dram_tensor(
        "idx_hi", [P, max(total_hi, 1) * 8], mybir.dt.int16, kind="ExternalInput"
    )
    m_dst16_d = nc.dram_tensor("m_dst16", [P, total_lo + total_hi], mybir.dt.float16, kind="ExternalInput")
    if pipe:
        idxg_d = [
            nc.dram_tensor(f"idx_g{h}", [P, max(tg[h], 1) * 8], mybir.dt.int16, kind="ExternalInput")
            for h in range(3)
        ]
        m_dst2_d = nc.dram_tensor(
            "m_dst16_2", [P, sum(tg)], mybir.dt.float16, kind="ExternalInput"
        )

    dinvd_d = nc.dram_tensor("dinvd", [P, NBLK], mybir.dt.float32, kind="ExternalInput")
    dinvd2_d = nc.dram_tensor("dinvd2", [P, NBLK], mybir.dt.float32, kind="ExternalInput")
    w1_d = nc.dram_tensor("w1", [IN_DIM, HID_DIM], mybir.dt.float16, kind="ExternalInput")
    w2_d = nc.dram_tensor("w2", [HID_DIM, OUT_DIM], mybir.dt.float16, kind="ExternalInput")
    b1_d = (
        nc.dram_tensor("b1rep", [P, HID_DIM], mybir.dt.float32, kind="ExternalInput")
        if meta["has_b1"]
        else None
    )
    b2_d = (
        nc.dram_tensor("b2rep", [P, OUT_DIM], mybir.dt.float32, kind="ExternalInput")
        if meta["has_b2"]
        else None
    )
    out_d = nc.dram_tensor("out", [SHARD, OUT_DIM], mybir.dt.float32, kind="ExternalOutput")

    with tile.TileContext(nc) as tc:
        with (
            tc.tile_pool(name="const", bufs=1) as const,
            tc.tile_pool(name="gpool", bufs=gbufs) as gpool,
            tc.tile_pool(name="selp", bufs=int(os.environ.get("GCN_SELP_BUFS", "16"))) as selp,
            tc.tile_pool(name="sbuf", bufs=3) as sbp,
            tc.tile_pool(name="accp", bufs=1) as accp,
            tc.tile_pool(name="psA", bufs=4, space="PSUM") as psA,
            tc.tile_pool(name="psB", bufs=2, space="PSUM") as psB,
            tc.tile_pool(name="dram", bufs=1, space="DRAM") as dram,
        ):
            # ---- one-time loads
            idx_lo_sb = const.tile([P, total_lo * 8], mybir.dt.int16)
            nc.sync.dma_start(out=idx_lo_sb[:], in_=idx_lo_d[:])
            idx_hi_sb = const.tile([P, max(total_hi, 1) * 8], mybir.dt.int16)
            nc.sync.dma_start(out=idx_hi_sb[:], in_=idx_hi_d[:])
            m_dst16_sb = const.tile([P, total_lo + total_hi], mybir.dt.float16)
            nc.sync.dma_start(out=m_dst16_sb[:], in_=m_dst16_d[:])
            if pipe:
                idxg_sb = []
                for h in range(3):
                    t = const.tile([P, max(tg[h], 1) * 8], mybir.dt.int16, name=f"idxg{h}")
                    nc.sync.dma_start(out=t[:], in_=idxg_d[h][:])
                    idxg_sb.append(t)
                m_dst2_sb = const.tile([P, sum(tg)], mybir.dt.float16)
                nc.sync.dma_start(out=m_dst2_sb[:], in_=m_dst2_d[:])

            dinvd_sb = const.tile([P, NBLK], mybir.dt.float32)
            nc.sync.dma_start(out=dinvd_sb[:], in_=dinvd_d[:])
            dinvd2_sb = const.tile([P, NBLK], mybir.dt.float32)
            nc.sync.dma_start(out=dinvd2_sb[:], in_=dinvd2_d[:])
            w1_sb = const.tile([IN_DIM, HID_DIM], mybir.dt.float16)
            nc.sync.dma_start(out=w1_sb[:], in_=w1_d[:])
            w2_sb = const.tile([HID_DIM, OUT_DIM], mybir.dt.float16)
            nc.sync.dma_start(out=w2_sb[:], in_=w2_d[:])
            b1_sb = b2_sb = None
            if b1_d is not None:
                b1_sb = const.tile([P, HID_DIM], mybir.dt.float32)
                nc.sync.dma_start(out=b1_sb[:], in_=b1_d[:])
            if b2_d is not None:
                b2_sb = const.tile([P, OUT_DIM], mybir.dt.float32)
                nc.sync.dma_start(out=b2_sb[:], in_=b2_d[:])

            iota32 = const.tile([P, P], mybir.dt.int32)
            nc.gpsimd.iota(iota32[:], pattern=[[1, P]], base=0, channel_multiplier=0)
            iota16 = const.tile([P, P], mybir.dt.float16)
            nc.vector.tensor_copy(out=iota16[:], in_=iota32[:])
            BW = 8
            iota16b = const.tile([P, BW, P], mybir.dt.float16)
            for g in range(BW):
                nc.vector.tensor_copy(out=iota16b[:, g, :], in_=iota16[:])

            if pipe:
                # per-group store targets (tile-granular DRAM dep tracking
                # makes each AllGather wait only on its own group's stores)
                h16sh_g = [
                    dram.tile([rows, HID_DIM], mybir.dt.float16, name=f"h16shg{gi}")
                    for gi, (r0, rows) in enumerate(groups)
                ]
                ag_out = [
                    dram.tile([NCORES * rows, HID_DIM], mybir.dt.float16,
                              addr_space="Shared", name=f"ag_out{gi}")
                    for gi, (r0, rows) in enumerate(groups)
                ]
            else:
                h16sh = dram.tile([SHARD, HID_DIM], mybir.dt.float16)
                h16full = dram.tile([N_NODES, HID_DIM], mybir.dt.float16, addr_space="Shared")

            # SWDGE descriptor rings can't hold a whole-superblock gather in
            # one instruction (ring carveout is O(512) descs/engine; the
            # ucode's await_space never succeeds past that) — split calls.
            MAXCH = int(os.environ.get("GCN_GATHER_CHUNKS", "16"))
            qrot = [0]

            def gather_split(dst_tile, src_ap, idx_sb, ch_off, n_ch, fin):
                for k0 in range(0, n_ch, MAXCH):
                    kn = min(MAXCH, n_ch - k0)
                    nc.gpsimd.dma_gather(
                        out_ap=dst_tile[:, k0 : k0 + kn, :],
                        in_ap=src_ap,
                        idxs_ap=idx_sb[:, (ch_off + k0) * 8 : (ch_off + k0 + kn) * 8],
                        num_idxs=kn * P,
                        num_idxs_reg=kn * P,
                        elem_size=fin,
                        single_packet=False,
                        queue_num=qrot[0] % 4,
                    )
                    qrot[0] += 1

            def layer(src_dram, fin, w_sb, fout, bias_sb, relu, sink,
                      cap_l, idx_lo_t, idx_hi_t, m_dst_t, total_lo_l,
                      sb_hook=None, sb_pre_hook=None):
                lo_off = 0          # lo chunk offset (also m_dst column)
                hi_off = 0
                for si, sb in enumerate(sbs):
                    n_lo = int(cap_l[sb, 0].sum())
                    n_hi = int(cap_l[sb, 1].sum())
                    glo = gpool.tile([P, n_lo, fin], mybir.dt.float16, tag="glo")
                    gather_split(glo, src_dram[:], idx_lo_t, lo_off, n_lo, fin)
                    ghi = None
                    if n_hi:
                        ghi = gpool.tile([P, n_hi, fin], mybir.dt.float16, tag="ghi")
                        gather_split(ghi, src_dram[SPLIT:, :], idx_hi_t, hi_off, n_hi, fin)
                    if sb_pre_hook is not None:
                        sb_pre_hook(si)
                    lo_c = 0
                    hi_c = 0
                    for b in sb:
                        # two contiguous chunk runs per block (lo then hi)
                        runs = []
                        if int(cap_l[b, 0]):
                            runs.append((glo, lo_c, lo_off + lo_c, int(cap_l[b, 0])))
                        if int(cap_l[b, 1]):
                            runs.append(
                                (ghi, hi_c, total_lo_l + hi_off + hi_c, int(cap_l[b, 1]))
                            )
                        lo_c += int(cap_l[b, 0])
                        hi_c += int(cap_l[b, 1])
                        total = sum(r[3] for r in runs)

                        ps_s = psA.tile([P, P], mybir.dt.float32, tag="psA")
                        jj = 0
                        for gt, gc0, mc0, cnt in runs:
                            for g0 in range(0, cnt, BW):
                                g = min(BW, cnt - g0)
                                selt = selp.tile([P, BW, P], mybir.dt.float16, tag="selb")
                                nc.vector.tensor_tensor(
                                    out=selt[:, :g, :],
                                    in0=m_dst_t[
                                        :, mc0 + g0 : mc0 + g0 + g
                                    ].to_broadcast([P, g, P]),
                                    in1=iota16b[:, :g, :],
                                    op=mybir.AluOpType.is_equal,
                                )
                                for k in range(g):
                                    nc.tensor.matmul(
                                        out=ps_s[:],
                                        lhsT=gt[:, gc0 + g0 + k, :],
                                        rhs=selt[:, k, :],
                                        start=(jj == 0),
                                        stop=(jj == total - 1),
                                    )
                                    jj += 1
                        sT = sbp.tile([P, P], mybir.dt.float16, tag="sT")
                        nc.vector.tensor_copy(out=sT[:], in_=ps_s[:])
                        ps_h = psB.tile([P, fout], mybir.dt.float32, tag="psB")
                        nc.tensor.matmul(
                            out=ps_h[:], lhsT=sT[:], rhs=w_sb[:], start=True, stop=True
                        )
                        sink(b, ps_h, bias_sb, relu)
                    if sb_hook is not None:
                        sb_hook(si)
                    lo_off += n_lo
                    hi_off += n_hi

            def store(dst_dram, dt, fout, extra_dinv):
                # layer 1 stores h16' = dinv * relu(dinv*z + b1) (the leading
                # dinv is the src-side prescale for layer 2's gather); with
                # b1 == 0 this folds to relu(dinv^2 * z) in one ACT op.
                def sink(b, ps_h, bias_sb, relu):
                    rows = P if b < NBLK - 1 else LAST_ROWS
                    o_t = sbp.tile([P, fout], dt, tag=f"o{dt}")
                    if bias_sb is None:
                        sc = dinvd2_sb if extra_dinv else dinvd_sb
                        nc.scalar.activation(
                            out=o_t[:],
                            in_=ps_h[:],
                            func=(
                                mybir.ActivationFunctionType.Relu
                                if relu
                                else mybir.ActivationFunctionType.Copy
                            ),
                            scale=sc[:, b : b + 1],
                        )
                    else:
                        t1 = sbp.tile([P, fout], mybir.dt.float32, tag="t1")
                        nc.vector.tensor_scalar(
                            out=t1[:],
                            in0=ps_h[:],
                            scalar1=dinvd_sb[:, b : b + 1],
                            scalar2=None,
                            op0=mybir.AluOpType.mult,
                        )
                        nc.vector.tensor_tensor(
                            out=t1[:], in0=t1[:], in1=bias_sb[:], op=mybir.AluOpType.add
                        )
                        if relu:
                            nc.scalar.activation(
                                out=o_t[:],
                                in_=t1[:],
                                func=mybir.ActivationFunctionType.Relu,
                                scale=(
                                    dinvd_sb[:, b : b + 1] if extra_dinv else 1.0
                                ),
                            )
                        elif extra_dinv:
                            nc.vector.tensor_scalar(
                                out=o_t[:],
                                in0=t1[:],
                                scalar1=dinvd_sb[:, b : b + 1],
                                scalar2=None,
                                op0=mybir.AluOpType.mult,
                            )
                        else:
                            nc.vector.tensor_copy(out=o_t[:], in_=t1[:])
                    if isinstance(dst_dram, list):
                        s = len(groups) - 1
                        while groups[s][0] > b * P:
                            s -= 1
                        r0 = b * P - groups[s][0]
                        tgt = dst_dram[s]
                    else:
                        r0 = b * P
                        tgt = dst_dram
                    nc.sync.dma_start(
                        out=tgt[r0 : r0 + rows, :], in_=o_t[:rows, :]
                    )

                return sink

            if pipe:
                # ---- 3-group exchange: each AllGather fires as soon as its
                # group's stores are in flight (group A while layer 1 still
                # gathers; B and C are interleaved into layer 2's gather
                # stream so the in-order pool queue reaches them only after
                # their store deps have cleared).  Layer 2 gathers straight
                # from the Shared tiles, one int16 index stream per group,
                # accumulating raw features per block in accA until the last
                # group, then applies W2 and stores.
                def fire_ag(gi):
                    nc.gpsimd.collective_compute(
                        "AllGather",
                        mybir.AluOpType.bypass,
                        replica_groups=[list(range(NCORES))],
                        ins=[h16sh_g[gi][:]],
                        outs=[ag_out[gi][:]],
                    )

                def sb_pre_hook(si):
                    if si == 3:
                        fire_ag(0)

                layer(x16, IN_DIM, w1_sb, HID_DIM, b1_sb, True,
                      store(h16sh_g, mybir.dt.float16, HID_DIM, True),
                      cap, idx_lo_sb, idx_hi_sb, m_dst16_sb, total_lo,
                      sb_pre_hook=sb_pre_hook)

                # ---- layer 2: 3 group passes; A copies into accA, B adds,
                # C adds + applies W2 + stores.
                accA = accp.tile([P, NBLK, P], mybir.dt.float16)
                sink2 = store(out_d, mybir.dt.float32, OUT_DIM, False)
                mbase = [0, tg[0], tg[0] + tg[1]]
                AG_AT = {(0, 1): 1, (1, 3): 2}   # (pass, sb) -> group to fire
                for g in range(3):
                    off_g = 0
                    for si, sb in enumerate(sbs):
                        n_g = int(cap2[sb, g].sum())
                        gt = None
                        if n_g:
                            gt = gpool.tile([P, n_g, HID_DIM], mybir.dt.float16, tag="glo")
                            gather_split(gt, ag_out[g][:], idxg_sb[g], off_g, n_g, HID_DIM)
                        if (g, si) in AG_AT:
                            fire_ag(AG_AT[(g, si)])
                        lo_c = 0
                        for b in sb:
                            cnt = int(cap2[b, g])
                            ps_s = None
                            if cnt:
                                ps_s = psA.tile([P, P], mybir.dt.float32, tag="psA")
                                jj = 0
                                for g0 in range(0, cnt, BW):
                                    gg = min(BW, cnt - g0)
                                    selt = selp.tile([P, BW, P], mybir.dt.float16, tag="selb")
                                    nc.vector.tensor_tensor(
                                        out=selt[:, :gg, :],
                                        in0=m_dst2_sb[
                                            :, mbase[g] + off_g + lo_c + g0 : mbase[g] + off_g + lo_c + g0 + gg
                                        ].to_broadcast([P, gg, P]),
                                        in1=iota16b[:, :gg, :],
                                        op=mybir.AluOpType.is_equal,
                                    )
                                    for k in range(gg):
                                        nc.tensor.matmul(
                                            out=ps_s[:],
                                            lhsT=gt[:, lo_c + g0 + k, :],
                                            rhs=selt[:, k, :],
                                            start=(jj == 0),
                                            stop=(jj == cnt - 1),
                                        )
                                        jj += 1
                            lo_c += cnt
                            if g == 0:
                                nc.vector.tensor_copy(out=accA[:, b, :], in_=ps_s[:])
                            elif g == 1:
                                if cnt:
                                    nc.vector.tensor_tensor(
                                        out=accA[:, b, :], in0=ps_s[:],
                                        in1=accA[:, b, :], op=mybir.AluOpType.add,
                                    )
                            else:
                                if cnt:
                                    sT = sbp.tile([P, P], mybir.dt.float16, tag="sT")
                                    nc.vector.tensor_tensor(
                                        out=sT[:], in0=ps_s[:],
                                        in1=accA[:, b, :], op=mybir.AluOpType.add,
                                    )
                                    lhsT = sT[:]
                                else:
                                    lhsT = accA[:, b, :]
                                ps_h = psB.tile([P, OUT_DIM], mybir.dt.float32, tag="psB")
                                nc.tensor.matmul(
                                    out=ps_h[:], lhsT=lhsT, rhs=w2_sb[:], start=True, stop=True
                                )
                                sink2(b, ps_h, b2_sb, False)
                        off_g += n_g
            else:
                # ---- layer 1: x16 -> h16sh (h16' = dinv * relu(...), the
                # src-side prescale for layer 2's gathers)
                layer(x16, IN_DIM, w1_sb, HID_DIM, b1_sb, True,
                      store(h16sh, mybir.dt.float16, HID_DIM, True),
                      cap, idx_lo_sb, idx_hi_sb, m_dst16_sb, total_lo)

                # ---- exchange (node-order table)
                nc.gpsimd.collective_compute(
                    "AllGather",
                    mybir.AluOpType.bypass,
                    replica_groups=[list(range(NCORES))],
                    ins=[h16sh[:]],
                    outs=[h16full[:]],
                )

                # ---- layer 2: h16full -> out (same edge structure as layer 1)
                layer(h16full, HID_DIM, w2_sb, OUT_DIM, b2_sb, False,
                      store(out_d, mybir.dt.float32, OUT_DIM, False),
                      cap, idx_lo_sb, idx_hi_sb, m_dst16_sb, total_lo)

    nc.compile()
    return nc


_CACHE = {}


def _enable_trace_shim():
    """This image's antenv lacks axon_hooks; recreate it so trace=True works,
    and stub the artifact upload (no bucket access here)."""
    import types

    try:
        import antenv.axon_hooks  # noqa: F401
    except ImportError:
        mod = types.ModuleType("antenv.axon_hooks")
        _h = [None]
        mod.set_axon_ntff_profile_hook = lambda h: _h.__setitem__(0, h)
        mod.get_axon_ntff_profile_hook = lambda: _h[0]
        sys.modules["antenv.axon_hooks"] = mod
        import antenv

        antenv.axon_hooks = mod
        from trn_agent_boot.trn_boot import _ntff_profile_via_ctypes

        mod.set_axon_ntff_profile_hook(
            _ntff_profile_via_ctypes("/opt/axon/libaxon_pjrt.so")
        )
    import concourse.bass_utils as bu

    bu.upload_artifacts = lambda tmpdir: tmpdir


def kernel(x, edge_index, W1, b1, W2, b2):
    global LAST_RESULTS
    meta, in_maps = preprocess(x, edge_index, W1, b1, W2, b2)
    key = (
        tuple(meta["cap"].reshape(-1)),
        tuple(meta["cap2"].reshape(-1)),
        meta["has_b1"],
        meta["has_b2"],
        os.environ.get("GCN_PIPE", "1"),
        os.environ.get("GCN_AG_LAG", "2"),
        os.environ.get("GCN_GPOOL_BUFS", "3"),
    )
    if key not in _CACHE:
        _CACHE[key] = build(meta)
    nc = _CACHE[key]
    trace = bool(int(os.environ.get("GCN_TRACE", "0")))
    if trace:
        _enable_trace_shim()
    res = run_bass_kernel_spmd(
        nc, in_maps, core_ids=list(range(NCORES)), trace=trace
    )
    LAST_RESULTS = res
    return np.concatenate([res.results[c]["out"] for c in range(NCORES)], axis=0)



# revision 43
# speedup vs baseline: 1.2171x; 1.2171x over previous
"""Two-layer GCN on 8 Trainium2 NeuronCores (Bass/Tile).

Math (reference, per layer):
    deg  = segment_sum(ones, dst)                 # target-side degrees
    dinv = where(deg>0, rsqrt(deg), 0)
    out[d] = dinv[d] * sum_{e: dst[e]=d} dinv[src[e]] * x[src[e]]  @ W  + b
(the x@W GEMM commutes with the segment sum, so we aggregate raw features
and apply W once per 128-node output block).

Distribution: dst nodes (and their incident edges) are sharded across the 8
cores; x (fp16) is replicated in every core's HBM.  Per 128-edge chunk the
kernel gathers the source rows with dma_gather, builds a dinv-weighted
one-hot selection matrix [128e x 128dst] on DVE, and scatter-adds via a
TensorE matmul accumulating in PSUM: psum[f, d] += gathered.T @ sel.  A
second matmul applies the layer weight; the layer-1 activations are
exchanged with an AllGather so layer 2 can gather any source row.

dma_gather indices are int16, so sources are split into lo (< 32768) and
hi (>= 32768) edge lists; the hi gather reads from a view of the feature
table offset by 32768 rows.
"""

import os
import sys
import time

sys.path.insert(0, "/opt/trn_rl_repo")

import numpy as np

import concourse.bass as bass
import concourse.bacc as bacc
import concourse.tile as tile
from concourse import mybir
from concourse.bass_utils import run_bass_kernel_spmd

P = 128
N_NODES = 50000
N_EDGES = 800000
IN_DIM = 128
HID_DIM = 128
OUT_DIM = 64
NCORES = 8
SHARD = N_NODES // NCORES          # 6250
NBLK = (SHARD + P - 1) // P        # 49 dst blocks per core (48 full + 106)
SPLIT = 32768                      # int16 index limit
SB_BLOCKS = 7                      # dst blocks per superblock (gather batch)
LAST_ROWS = SHARD - (NBLK - 1) * P # rows in the final dst block

# Filled by kernel() on the last run (for test.py introspection).
LAST_RESULTS = None


# --------------------------------------------------------------------------
# Host-side preprocessing
# --------------------------------------------------------------------------

def _wrap_idx(idx_chunks):
    """int16 indices for one dma_gather call: [16, n/16] wrap replicated to
    128 partitions.  idx_chunks: int array [n_chunks, 128]."""
    flat = idx_chunks.reshape(-1)
    n = flat.shape[0]
    arr = flat.reshape(n // 16, 16).T.astype(np.int16)   # [16, n/16]
    return np.tile(arr, (8, 1))                          # [128, n/16]


def preprocess(x, edge_index, W1, b1, W2, b2):
    x = np.asarray(x, dtype=np.float32)
    edge_index = np.asarray(edge_index).astype(np.int64)
    src_g = edge_index[0].astype(np.int32)
    dst_g = edge_index[1].astype(np.int32)

    deg = np.bincount(dst_g, minlength=N_NODES).astype(np.float32)
    dinv = np.where(deg > 0, 1.0 / np.sqrt(np.maximum(deg, 1.0)), 0.0).astype(
        np.float32
    )

    # per (core, block, lo/hi) edge lists
    owner = dst_g // SHARD
    blk_loc = (dst_g % SHARD) // P
    rel = (dst_g % SHARD) % P

    # superblock structure (gather batching) + the 3-group exchange layout:
    # layer-1 activations are exchanged with three AllGathers over superblock
    # groups [0-1], [2-4], [5-6], each landing in its own Shared tile while
    # later superblocks still compute.  Each group spans < 32768 rows of the
    # [group][core][row] permuted table, so layer 2 gathers straight from the
    # Shared tiles with one int16 index stream per group (no lo/hi split).
    sbs = [list(range(s, min(s + SB_BLOCKS, NBLK))) for s in range(0, NBLK, SB_BLOCKS)]
    sb_rows = []
    for sb in sbs:
        r0 = sb[0] * P
        r1 = min(sb[-1] * P + P, SHARD)
        sb_rows.append((r0, r1 - r0))
    GROUP_SBS = [(0, 6), (6, 7)]   # sb index ranges per group
    groups = []
    for (s0, s1) in GROUP_SBS:
        r0 = sb_rows[s0][0]
        r1 = sb_rows[s1 - 1][0] + sb_rows[s1 - 1][1]
        groups.append((r0, r1 - r0))
    pos_base = np.cumsum([0] + [NCORES * rows for (_, rows) in groups])[:-1]
    # pos[n] for n = c*SHARD + r, group-major then core-major
    pos = np.empty(N_NODES, np.int64)
    base = 0
    for (r0, rows) in groups:
        for c in range(NCORES):
            n0 = c * SHARD + r0
            pos[n0 : n0 + rows] = np.arange(base + c * rows, base + (c + 1) * rows)
        base += NCORES * rows

    def make_buckets(src_key, binof, nbins):
        key = ((owner * NBLK + blk_loc) * nbins + binof).astype(np.int64)
        order = np.argsort(key, kind="stable")
        key_s = key[order]
        src_s = src_key[order]
        rel_s = rel[order]
        bounds = np.searchsorted(key_s, np.arange(NCORES * NBLK * nbins + 1))
        nchunks = (
            (bounds[1:] - bounds[:-1]).reshape(NCORES, NBLK, nbins) + P - 1
        ) // P
        cap = nchunks.max(axis=0)
        cap[:, 0] = np.maximum(cap[:, 0], 1)
        return (src_s, rel_s, bounds), cap

    bk1, cap = make_buckets(src_g, (src_g >= SPLIT).astype(np.int32), 2)
    pos_src = pos[src_g].astype(np.int32)
    bk2, cap2 = make_buckets(pos_src, (pos_src >= SPLIT).astype(np.int32), 2)

    meta = {
        "cap": cap,
        "cap2": cap2,
        "sbs": sbs,
        "sb_rows": sb_rows,
        "groups": groups,
        "group_sbs": GROUP_SBS,
        "pos_base": pos_base,
        "has_b1": bool(np.any(np.asarray(b1))),
        "has_b2": bool(np.any(np.asarray(b2))),
    }

    # per-core arrays.  x is pre-scaled by dinv[src] so the selection matrix
    # is a plain one-hot (single DVE op); padded lanes get dstrel=-1, which
    # matches no iota column and therefore contributes nothing.
    in_maps = []
    x16 = (dinv[:, None] * x).astype(np.float16)

    def core_arrays(c, bk, cap_l, bases):
        src_s, rel_s, bounds = bk
        H = cap_l.shape[1]
        totals = [int(cap_l[:, h].sum()) for h in range(H)]
        cum = [0]
        for t in totals:
            cum.append(cum[-1] + t)
        idx_arr = [np.zeros((totals[h], P), np.int32) for h in range(H)]
        m_dst = np.zeros((P, cum[-1]), np.float32)
        off = [0] * H
        for b in range(NBLK):
            for h in range(H):
                k = (c * NBLK + b) * H + h
                s_arr = src_s[bounds[k] : bounds[k + 1]]
                r_arr = rel_s[bounds[k] : bounds[k + 1]]
                n = s_arr.shape[0]
                ncap = int(cap_l[b, h])
                idxs = np.zeros(ncap * P, np.int32)
                idxs[:n] = s_arr - bases[h]
                d_arr = np.full(ncap * P, -1.0, np.float32)
                d_arr[:n] = r_arr
                o = off[h]
                idx_arr[h][o : o + ncap] = idxs.reshape(ncap, P)
                m_dst[:, cum[h] + o : cum[h] + o + ncap] = d_arr.reshape(ncap, P).T
                off[h] += ncap
        wrapped = []
        for h in range(H):
            o = 0
            cols = []
            for sb in sbs:
                n_h = int(cap_l[sb, h].sum())
                if n_h:
                    cols.append(_wrap_idx(idx_arr[h][o : o + n_h]))
                    o += n_h
            wrapped.append(
                np.concatenate(cols, axis=1) if cols else np.zeros((P, 8), np.int16)
            )
        return wrapped, m_dst.astype(np.float16)

    for c in range(NCORES):
        (idx_lo_w, idx_hi_w), m_dst16 = core_arrays(c, bk1, cap, [0, SPLIT])
        idx2_w, m_dst16_2 = core_arrays(c, bk2, cap2, [0, SPLIT])

        tmp = np.zeros(NBLK * P, np.float32)
        tmp[:SHARD] = dinv[c * SHARD : (c + 1) * SHARD]
        dinvd = tmp.reshape(NBLK, P).T.copy()   # [p, b] = dinv[c*SHARD + b*P + p]

        im = {
            "x16": x16,
            "idx_lo": idx_lo_w,
            "idx_hi": idx_hi_w,
            "idx_g0": idx2_w[0],
            "idx_g1": idx2_w[1],
            "m_dst16": m_dst16,
            "m_dst16_2": m_dst16_2,
            "dinvd": dinvd,
            "dinvd2": dinvd * dinvd,
            "w1": np.asarray(W1, np.float32).astype(np.float16),
            "w2": np.asarray(W2, np.float32).astype(np.float16),
        }
        if meta["has_b1"]:
            im["b1rep"] = np.tile(np.asarray(b1, np.float32)[None, :], (P, 1))
        if meta["has_b2"]:
            im["b2rep"] = np.tile(np.asarray(b2, np.float32)[None, :], (P, 1))
        in_maps.append(im)
    return meta, in_maps


# --------------------------------------------------------------------------
# Bass kernel
# --------------------------------------------------------------------------

def build(meta):
    cap = meta["cap"]
    cap2 = meta["cap2"]
    sbs = meta["sbs"]
    sb_rows = meta["sb_rows"]
    groups = meta["groups"]
    total_lo = int(cap[:, 0].sum())
    total_hi = int(cap[:, 1].sum())
    tg = [int(cap2[:, h].sum()) for h in range(cap2.shape[1])]

    nc = bacc.Bacc(
        "TRN2",
        target_bir_lowering=False,
        debug=False,
        enable_asserts=True,
        num_devices=NCORES,
        num_swdge_queues=4,
        dynamic_dma_scratch_size=int(os.environ.get("GCN_DMA_SCRATCH", "16384")),
    )
    pipe = bool(int(os.environ.get("GCN_PIPE", "1")))
    ag_lag = int(os.environ.get("GCN_AG_LAG", "2"))
    gbufs = int(os.environ.get("GCN_GPOOL_BUFS", "3"))
    x16 = nc.dram_tensor("x16", [N_NODES, IN_DIM], mybir.dt.float16, kind="ExternalInput")
    idx_lo_d = nc.dram_tensor("idx_lo", [P, total_lo * 8], mybir.dt.int16, kind="ExternalInput")
    idx_hi_d = nc.dram_tensor(
        "idx_hi", [P, max(total_hi, 1) * 8], mybir.dt.int16, kind="ExternalInput"
    )
    m_dst16_d = nc.dram_tensor("m_dst16", [P, total_lo + total_hi], mybir.dt.float16, kind="ExternalInput")
    if pipe:
        idxg_d = [
            nc.dram_tensor(f"idx_g{h}", [P, max(tg[h], 1) * 8], mybir.dt.int16, kind="ExternalInput")
            for h in range(2)
        ]
        m_dst2_d = nc.dram_tensor(
            "m_dst16_2", [P, sum(tg)], mybir.dt.float16, kind="ExternalInput"
        )

    dinvd_d = nc.dram_tensor("dinvd", [P, NBLK], mybir.dt.float32, kind="ExternalInput")
    dinvd2_d = nc.dram_tensor("dinvd2", [P, NBLK], mybir.dt.float32, kind="ExternalInput")
    w1_d = nc.dram_tensor("w1", [IN_DIM, HID_DIM], mybir.dt.float16, kind="ExternalInput")
    w2_d = nc.dram_tensor("w2", [HID_DIM, OUT_DIM], mybir.dt.float16, kind="ExternalInput")
    b1_d = (
        nc.dram_tensor("b1rep", [P, HID_DIM], mybir.dt.float32, kind="ExternalInput")
        if meta["has_b1"]
        else None
    )
    b2_d = (
        nc.dram_tensor("b2rep", [P, OUT_DIM], mybir.dt.float32, kind="ExternalInput")
        if meta["has_b2"]
        else None
    )
    out_d = nc.dram_tensor("out", [SHARD, OUT_DIM], mybir.dt.float32, kind="ExternalOutput")

    with tile.TileContext(nc) as tc:
        with (
            tc.tile_pool(name="const", bufs=1) as const,
            tc.tile_pool(name="gpool", bufs=gbufs) as gpool,
            tc.tile_pool(name="selp", bufs=int(os.environ.get("GCN_SELP_BUFS", "16"))) as selp,
            tc.tile_pool(name="sbuf", bufs=3) as sbp,
            tc.tile_pool(name="accp", bufs=1) as accp,
            tc.tile_pool(name="psA", bufs=4, space="PSUM") as psA,
            tc.tile_pool(name="psB", bufs=2, space="PSUM") as psB,
            tc.tile_pool(name="dram", bufs=1, space="DRAM") as dram,
        ):
            # ---- one-time loads
            idx_lo_sb = const.tile([P, total_lo * 8], mybir.dt.int16)
            nc.sync.dma_start(out=idx_lo_sb[:], in_=idx_lo_d[:])
            idx_hi_sb = const.tile([P, max(total_hi, 1) * 8], mybir.dt.int16)
            nc.sync.dma_start(out=idx_hi_sb[:], in_=idx_hi_d[:])
            m_dst16_sb = const.tile([P, total_lo + total_hi], mybir.dt.float16)
            nc.sync.dma_start(out=m_dst16_sb[:], in_=m_dst16_d[:])
            if pipe:
                idxg_sb = []
                for h in range(2):
                    t = const.tile([P, max(tg[h], 1) * 8], mybir.dt.int16, name=f"idxg{h}")
                    nc.sync.dma_start(out=t[:], in_=idxg_d[h][:])
                    idxg_sb.append(t)
                m_dst2_sb = const.tile([P, sum(tg)], mybir.dt.float16)
                nc.sync.dma_start(out=m_dst2_sb[:], in_=m_dst2_d[:])

            dinvd_sb = const.tile([P, NBLK], mybir.dt.float32)
            nc.sync.dma_start(out=dinvd_sb[:], in_=dinvd_d[:])
            dinvd2_sb = const.tile([P, NBLK], mybir.dt.float32)
            nc.sync.dma_start(out=dinvd2_sb[:], in_=dinvd2_d[:])
            w1_sb = const.tile([IN_DIM, HID_DIM], mybir.dt.float16)
            nc.sync.dma_start(out=w1_sb[:], in_=w1_d[:])
            w2_sb = const.tile([HID_DIM, OUT_DIM], mybir.dt.float16)
            nc.sync.dma_start(out=w2_sb[:], in_=w2_d[:])
            b1_sb = b2_sb = None
            if b1_d is not None:
                b1_sb = const.tile([P, HID_DIM], mybir.dt.float32)
                nc.sync.dma_start(out=b1_sb[:], in_=b1_d[:])
            if b2_d is not None:
                b2_sb = const.tile([P, OUT_DIM], mybir.dt.float32)
                nc.sync.dma_start(out=b2_sb[:], in_=b2_d[:])

            iota32 = const.tile([P, P], mybir.dt.int32)
            nc.gpsimd.iota(iota32[:], pattern=[[1, P]], base=0, channel_multiplier=0)
            iota16 = const.tile([P, P], mybir.dt.float16)
            nc.vector.tensor_copy(out=iota16[:], in_=iota32[:])
            BW = 8
            iota16b = const.tile([P, BW, P], mybir.dt.float16)
            for g in range(BW):
                nc.vector.tensor_copy(out=iota16b[:, g, :], in_=iota16[:])

            if pipe:
                # per-group store targets (tile-granular DRAM dep tracking
                # makes each AllGather wait only on its own group's stores)
                h16sh_g = [
                    dram.tile([rows, HID_DIM], mybir.dt.float16, name=f"h16shg{gi}")
                    for gi, (r0, rows) in enumerate(groups)
                ]
                ag_out = [
                    dram.tile([NCORES * rows, HID_DIM], mybir.dt.float16,
                              addr_space="Shared", name=f"ag_out{gi}")
                    for gi, (r0, rows) in enumerate(groups)
                ]
            else:
                h16sh = dram.tile([SHARD, HID_DIM], mybir.dt.float16)
                h16full = dram.tile([N_NODES, HID_DIM], mybir.dt.float16, addr_space="Shared")

            # SWDGE descriptor rings can't hold a whole-superblock gather in
            # one instruction (ring carveout is O(512) descs/engine; the
            # ucode's await_space never succeeds past that) — split calls.
            MAXCH = int(os.environ.get("GCN_GATHER_CHUNKS", "16"))
            qrot = [0]

            def gather_split(dst_tile, src_ap, idx_sb, ch_off, n_ch, fin):
                for k0 in range(0, n_ch, MAXCH):
                    kn = min(MAXCH, n_ch - k0)
                    nc.gpsimd.dma_gather(
                        out_ap=dst_tile[:, k0 : k0 + kn, :],
                        in_ap=src_ap,
                        idxs_ap=idx_sb[:, (ch_off + k0) * 8 : (ch_off + k0 + kn) * 8],
                        num_idxs=kn * P,
                        num_idxs_reg=kn * P,
                        elem_size=fin,
                        single_packet=False,
                        queue_num=qrot[0] % 4,
                    )
                    qrot[0] += 1

            def layer(src_dram, fin, w_sb, fout, bias_sb, relu, sink,
                      cap_l, idx_lo_t, idx_hi_t, m_dst_t, total_lo_l,
                      sb_hook=None, sb_pre_hook=None):
                lo_off = 0          # lo chunk offset (also m_dst column)
                hi_off = 0
                for si, sb in enumerate(sbs):
                    n_lo = int(cap_l[sb, 0].sum())
                    n_hi = int(cap_l[sb, 1].sum())
                    glo = gpool.tile([P, n_lo, fin], mybir.dt.float16, tag="glo")
                    gather_split(glo, src_dram[:], idx_lo_t, lo_off, n_lo, fin)
                    ghi = None
                    if n_hi:
                        ghi = gpool.tile([P, n_hi, fin], mybir.dt.float16, tag="ghi")
                        gather_split(ghi, src_dram[SPLIT:, :], idx_hi_t, hi_off, n_hi, fin)
                    if sb_pre_hook is not None:
                        sb_pre_hook(si)
                    lo_c = 0
                    hi_c = 0
                    for b in sb:
                        # two contiguous chunk runs per block (lo then hi)
                        runs = []
                        if int(cap_l[b, 0]):
                            runs.append((glo, lo_c, lo_off + lo_c, int(cap_l[b, 0])))
                        if int(cap_l[b, 1]):
                            runs.append(
                                (ghi, hi_c, total_lo_l + hi_off + hi_c, int(cap_l[b, 1]))
                            )
                        lo_c += int(cap_l[b, 0])
                        hi_c += int(cap_l[b, 1])
                        total = sum(r[3] for r in runs)

                        ps_s = psA.tile([P, P], mybir.dt.float32, tag="psA")
                        jj = 0
                        for gt, gc0, mc0, cnt in runs:
                            for g0 in range(0, cnt, BW):
                                g = min(BW, cnt - g0)
                                selt = selp.tile([P, BW, P], mybir.dt.float16, tag="selb")
                                nc.vector.tensor_tensor(
                                    out=selt[:, :g, :],
                                    in0=m_dst_t[
                                        :, mc0 + g0 : mc0 + g0 + g
                                    ].to_broadcast([P, g, P]),
                                    in1=iota16b[:, :g, :],
                                    op=mybir.AluOpType.is_equal,
                                )
                                for k in range(g):
                                    nc.tensor.matmul(
                                        out=ps_s[:],
                                        lhsT=gt[:, gc0 + g0 + k, :],
                                        rhs=selt[:, k, :],
                                        start=(jj == 0),
                                        stop=(jj == total - 1),
                                    )
                                    jj += 1
                        sT = sbp.tile([P, P], mybir.dt.float16, tag="sT")
                        nc.vector.tensor_copy(out=sT[:], in_=ps_s[:])
                        ps_h = psB.tile([P, fout], mybir.dt.float32, tag="psB")
                        nc.tensor.matmul(
                            out=ps_h[:], lhsT=sT[:], rhs=w_sb[:], start=True, stop=True
                        )
                        sink(b, ps_h, bias_sb, relu)
                    if sb_hook is not None:
                        sb_hook(si)
                    lo_off += n_lo
                    hi_off += n_hi

            def store(dst_dram, dt, fout, extra_dinv):
                # layer 1 stores h16' = dinv * relu(dinv*z + b1) (the leading
                # dinv is the src-side prescale for layer 2's gather); with
                # b1 == 0 this folds to relu(dinv^2 * z) in one ACT op.
                def sink(b, ps_h, bias_sb, relu):
                    rows = P if b < NBLK - 1 else LAST_ROWS
                    o_t = sbp.tile([P, fout], dt, tag=f"o{dt}")
                    if bias_sb is None:
                        sc = dinvd2_sb if extra_dinv else dinvd_sb
                        nc.scalar.activation(
                            out=o_t[:],
                            in_=ps_h[:],
                            func=(
                                mybir.ActivationFunctionType.Relu
                                if relu
                                else mybir.ActivationFunctionType.Copy
                            ),
                            scale=sc[:, b : b + 1],
                        )
                    else:
                        t1 = sbp.tile([P, fout], mybir.dt.float32, tag="t1")
                        nc.vector.tensor_scalar(
                            out=t1[:],
                            in0=ps_h[:],
                            scalar1=dinvd_sb[:, b : b + 1],
                            scalar2=None,
                            op0=mybir.AluOpType.mult,
                        )
                        nc.vector.tensor_tensor(
                            out=t1[:], in0=t1[:], in1=bias_sb[:], op=mybir.AluOpType.add
                        )
                        if relu:
                            nc.scalar.activation(
                                out=o_t[:],
                                in_=t1[:],
                                func=mybir.ActivationFunctionType.Relu,
                                scale=(
                                    dinvd_sb[:, b : b + 1] if extra_dinv else 1.0
                                ),
                            )
                        elif extra_dinv:
                            nc.vector.tensor_scalar(
                                out=o_t[:],
                                in0=t1[:],
                                scalar1=dinvd_sb[:, b : b + 1],
                                scalar2=None,
                                op0=mybir.AluOpType.mult,
                            )
                        else:
                            nc.vector.tensor_copy(out=o_t[:], in_=t1[:])
                    if isinstance(dst_dram, list):
                        s = len(groups) - 1
                        while groups[s][0] > b * P:
                            s -= 1
                        r0 = b * P - groups[s][0]
                        tgt = dst_dram[s]
                    else:
                        r0 = b * P
                        tgt = dst_dram
                    nc.sync.dma_start(
                        out=tgt[r0 : r0 + rows, :], in_=o_t[:rows, :]
                    )

                return sink

            if pipe:
                # ---- 2-group exchange: AG0 (superblocks 0..NSB-2, ~96% of
                # rows) fires right after the last superblock's gathers are
                # issued, so its transfer overlaps the L1 compute tail; the
                # tiny AG1 fires after the layer.  Each AllGather lands in a
                # Shared tile and is copied contiguously (group-major
                # permuted order) into the local table layer 2 gathers from;
                # the copies ride the idle scalar HWDGE queue.
                nsb = len(sbs)
                h16loc = dram.tile([N_NODES, HID_DIM], mybir.dt.float16)

                def fire_ag(gi):
                    r0, rows = groups[gi]
                    base = NCORES * r0
                    nc.gpsimd.collective_compute(
                        "AllGather",
                        mybir.AluOpType.bypass,
                        replica_groups=[list(range(NCORES))],
                        ins=[h16sh_g[gi][:]],
                        outs=[ag_out[gi][:]],
                    )
                    nc.scalar.dma_start(
                        out=h16loc[base : base + NCORES * rows, :],
                        in_=ag_out[gi][:],
                    )

                def sb_pre_hook(si):
                    if si == nsb - 1:
                        fire_ag(0)

                layer(x16, IN_DIM, w1_sb, HID_DIM, b1_sb, True,
                      store(h16sh_g, mybir.dt.float16, HID_DIM, True),
                      cap, idx_lo_sb, idx_hi_sb, m_dst16_sb, total_lo,
                      sb_pre_hook=sb_pre_hook)
                fire_ag(1)

                # ---- layer 2: permuted-table edge structure (cap2/idx2)
                layer(h16loc, HID_DIM, w2_sb, OUT_DIM, b2_sb, False,
                      store(out_d, mybir.dt.float32, OUT_DIM, False),
                      cap2, idxg_sb[0], idxg_sb[1], m_dst2_sb, tg[0])
            else:
                # ---- layer 1: x16 -> h16sh (h16' = dinv * relu(...), the
                # src-side prescale for layer 2's gathers)
                layer(x16, IN_DIM, w1_sb, HID_DIM, b1_sb, True,
                      store(h16sh, mybir.dt.float16, HID_DIM, True),
                      cap, idx_lo_sb, idx_hi_sb, m_dst16_sb, total_lo)

                # ---- exchange (node-order table)
                nc.gpsimd.collective_compute(
                    "AllGather",
                    mybir.AluOpType.bypass,
                    replica_groups=[list(range(NCORES))],
                    ins=[h16sh[:]],
                    outs=[h16full[:]],
                )

                # ---- layer 2: h16full -> out (same edge structure as layer 1)
                layer(h16full, HID_DIM, w2_sb, OUT_DIM, b2_sb, False,
                      store(out_d, mybir.dt.float32, OUT_DIM, False),
                      cap, idx_lo_sb, idx_hi_sb, m_dst16_sb, total_lo)

    nc.compile()
    return nc


_CACHE = {}


def _enable_trace_shim():
    """This image's antenv lacks axon_hooks; recreate it so trace=True works,
    and stub the artifact upload (no bucket access here)."""
    import types

    try:
        import antenv.axon_hooks  # noqa: F401
    except ImportError:
        mod = types.ModuleType("antenv.axon_hooks")
        _h = [None]
        mod.set_axon_ntff_profile_hook = lambda h: _h.__setitem__(0, h)
        mod.get_axon_ntff_profile_hook = lambda: _h[0]
        sys.modules["antenv.axon_hooks"] = mod
        import antenv

        antenv.axon_hooks = mod
        from trn_agent_boot.trn_boot import _ntff_profile_via_ctypes

        mod.set_axon_ntff_profile_hook(
            _ntff_profile_via_ctypes("/opt/axon/libaxon_pjrt.so")
        )
    import concourse.bass_utils as bu

    bu.upload_artifacts = lambda tmpdir: tmpdir


def kernel(x, edge_index, W1, b1, W2, b2):
    global LAST_RESULTS
    meta, in_maps = preprocess(x, edge_index, W1, b1, W2, b2)
    key = (
        tuple(meta["cap"].reshape(-1)),
        tuple(meta["cap2"].reshape(-1)),
        meta["has_b1"],
        meta["has_b2"],
        os.environ.get("GCN_PIPE", "1"),
        os.environ.get("GCN_AG_LAG", "2"),
        os.environ.get("GCN_GPOOL_BUFS", "3"),
    )
    if key not in _CACHE:
        _CACHE[key] = build(meta)
    nc = _CACHE[key]
    trace = bool(int(os.environ.get("GCN_TRACE", "0")))
    if trace:
        _enable_trace_shim()
    res = run_bass_kernel_spmd(
        nc, in_maps, core_ids=list(range(NCORES)), trace=trace
    )
    LAST_RESULTS = res
    return np.concatenate([res.results[c]["out"] for c in range(NCORES)], axis=0)

